# revision 46
# baseline (speedup 1.0000x reference)
"""Trainium2 Bass kernel for nn_AttentionOnDetail (dense transformer attention).

Sharding: head-parallel tensor parallelism across 8 NeuronCores.
Each core computes ONE attention head for all 4 batches using its slice of
W_qkvg (column-parallel) and W_out (row-parallel); the host sums the 8
partial outputs (the row-parallel all-reduce, done on host at gather time).

v2 per-core kernel structure (per batch):
  P1: q,k projection (optionally fp8 DoubleRow: 2 k-tiles of emb contracted
     per instruction at 0.5 cy/row); PSUM evacuated to bf16 SBUF on Pool;
     RoPE on DVE at 2x from bf16 SBUF; squares on DVE bf16; per-position
     sumsq via tiny N=1 ones-matmuls into columns; quake rsqrt on DVE
     columns; both factor rows scattered to DRAM bf16 and broadcast-loaded
     (scalar queue); q AND k scaled on DVE bf16 at 2x.
  P2: v,g projection (optionally fp8 DoubleRow); v evacuated into kt-PAIR
     tiles [128,2,258] (two ones-cols carry the softmax denominator),
     optionally fp8; sigmoid via tanh on ACT.
  Attention: S^T = k~^T q~ in bf16 per kt; causal masking PRE-exp by a
     third matmul accumulating a -1e30 mask block (master-slice trick) so
     exp zeroes above-diagonal blocks exactly; one exp per kt-PAIR with
     constant bias -2 (keeps exp in fp8 range; numerator and denominator
     share the bias so it cancels); PV optionally fp8 DoubleRow over kt
     pairs; epilogue fuses 1/denominator + sigmoid gating (DVE),
     PE-transposes y and projects out; o_sb/ygt evacuations on Pool.

Emission is unit-interleaved: attention(b-1) units are merged between
P1(b)/P2(b) units so every engine queue alternates between streams.
"""

import sys
import os

sys.path.insert(0, "/opt/trn_rl_repo")

import numpy as np
import ml_dtypes
from contextlib import ExitStack
from dataclasses import dataclass

import concourse.bass as bass
import concourse.bacc as bacc
import concourse.tile as tile
from concourse import mybir

F32 = mybir.dt.float32
F32R = mybir.dt.float32r
BF16 = mybir.dt.bfloat16
FP8 = mybir.dt.float8e4
AF = mybir.ActivationFunctionType
ALU = mybir.AluOpType
DR = mybir.MatmulPerfMode.DoubleRow

N_CORES = 8
N_EMBD = 256
N_HEAD = 8
HEAD_DIM = 256
P = 128  # partitions

NEG = -1.0e30
# exp(S - EXP_BIAS) must stay below fp8e4m3(IEEE) max 240; S_max ~ 7.74
EXP_BIAS = 2.5
W_SCALE = 32.0      # host pre-scale of W_qkvg when proj_fp8 (fp8 range)
V_SCALE = 4.0       # extra scale on stored v (fp8 range); folded into wot


@dataclass(frozen=True)
class Cfg:
    nb: int = 4          # batches per core
    t_real: int = 2048   # real tokens (output rows per batch)
    seq: int = 2176      # padded seq (sink + t_real padded to mult of 128)
    reps: int = 1        # repeat whole kernel body (timing-slope builds)
    proj_fp8: bool = False  # fp8 DoubleRow qkvg projections (too lossy)
    pv_fp8: bool = False    # fp8 pt/v + DoubleRow PV (too lossy)
    vg_fp8: bool = False    # fp8 DoubleRow v,g projection only (too lossy:
                            # peaked softmax rows don't average v quant away)
    # pool sizing knobs
    x0_bufs: int = 2
    qk_bufs: int = 6
    v_bufs: int = 28
    g_bufs: int = 52
    pt_bufs: int = 3
    depth2: bool = False  # interleave attention(b-2) instead of (b-1)

    @property
    def nt(self):
        return self.seq // P


FULL = Cfg()


def _chunks(total, width):
    out = []
    off = 0
    while off < total:
        w = min(width, total - off)
        out.append((off, w))
        off += w
    return out


def build_program(cfg: Cfg, cq2: float, shared_cossin: bool, ck2: float):
    """Build the single-core Bass program (same program runs on all 8 cores;
    per-core differences enter only through input data).

    cq2: effective c^2 for q's rsqrt (tao0/sqrt(hd))^2.
    ck2: effective c^2 for k's rsqrt (tao1^2).
    """
    nc = bacc.Bacc("TRN2", target_bir_lowering=False, debug=False)

    nb, seq, nt = cfg.nb, cfg.seq, cfg.nt
    t_real = cfg.t_real
    xdt = FP8 if cfg.proj_fp8 else BF16
    vgdt = FP8 if (cfg.proj_fp8 or cfg.vg_fp8) else BF16
    pvdt = FP8 if cfg.pv_fp8 else BF16
    eps_mult = (W_SCALE * W_SCALE) if cfg.proj_fp8 else 1.0

    # ---- DRAM I/O ----
    x0t = nc.dram_tensor("x0t", [nb, P, 2, seq], xdt, kind="ExternalInput").ap()
    if cfg.vg_fp8 and not cfg.proj_fp8:
        x0f = nc.dram_tensor(
            "x0f", [nb, P, 2, seq], FP8, kind="ExternalInput"
        ).ap()
    w1t = nc.dram_tensor("w1t", [P, 2, 512], xdt, kind="ExternalInput").ap()
    w2t = nc.dram_tensor("w2t", [P, 2, 512], vgdt, kind="ExternalInput").ap()
    wot = nc.dram_tensor("wot", [2, P, N_EMBD], BF16, kind="ExternalInput").ap()
    cosq = nc.dram_tensor("cosq", [P, seq], BF16, kind="ExternalInput").ap()
    sinq = nc.dram_tensor("sinq", [P, seq], BF16, kind="ExternalInput").ap()
    if not shared_cossin:
        cosk = nc.dram_tensor("cosk", [P, seq], BF16, kind="ExternalInput").ap()
        sink = nc.dram_tensor("sink", [P, seq], BF16, kind="ExternalInput").ap()
    # mask master: [NEG(128) | tri-neg(128) | zeros(128)]
    maskm = nc.dram_tensor("maskm", [P, 384], BF16, kind="ExternalInput").ap()
    onesc = nc.dram_tensor("onesc", [P, 2], BF16, kind="ExternalInput").ap()
    ident = nc.dram_tensor("ident", [P, P], BF16, kind="ExternalInput").ap()
    out = nc.dram_tensor("out", [nb, t_real, N_EMBD], F32, kind="ExternalOutput").ap()
    # DRAM scratch for the partition-broadcast of the q/k norm rows (bf16)
    rq_dram = nc.dram_tensor("rq_scratch", [nb, 2, seq], BF16).ap()

    eps = float(np.finfo(np.float32).eps) * eps_mult

    with tile.TileContext(nc) as tc, ExitStack() as ctx:
        consts = ctx.enter_context(tc.tile_pool(name="consts", bufs=1))
        x0p = ctx.enter_context(tc.tile_pool(name="x0", bufs=cfg.x0_bufs))
        qkp = ctx.enter_context(tc.tile_pool(name="qk", bufs=cfg.qk_bufs))
        rawp = ctx.enter_context(tc.tile_pool(name="raw", bufs=4))
        sqp = ctx.enter_context(tc.tile_pool(name="sq", bufs=4))
        rqp = ctx.enter_context(tc.tile_pool(name="rqb", bufs=4))
        vp = ctx.enter_context(tc.tile_pool(name="v", bufs=cfg.v_bufs))
        gp = ctx.enter_context(tc.tile_pool(name="g", bufs=cfg.g_bufs))
        ptp = ctx.enter_context(tc.tile_pool(name="pt", bufs=cfg.pt_bufs))
        yp = ctx.enter_context(tc.tile_pool(name="y", bufs=2))
        ytp = ctx.enter_context(tc.tile_pool(name="yt", bufs=2))
        outp = ctx.enter_context(tc.tile_pool(name="outs", bufs=2))
        smallp = ctx.enter_context(tc.tile_pool(name="small", bufs=3))
        psp = ctx.enter_context(tc.tile_pool(name="psp", bufs=2, space="PSUM"))

        def rsqrt_dve(dst_bf, u, tmp, tmp2):
            """dst_bf (bf16) = 1/sqrt(u); quake int guess + 2 Newton iters.
            u, tmp, tmp2 are f32 tiles of the same shape."""
            I32 = mybir.dt.int32
            di, ui = tmp2.bitcast(I32), u.bitcast(I32)
            nc.vector.tensor_scalar(di, ui, 1, None, ALU.logical_shift_right)
            nc.vector.tensor_scalar(di, di, -1, None, ALU.bitwise_xor)
            nc.vector.tensor_scalar(di, di, 0x5F3759E0, None, ALU.add)
            for it in range(2):
                nc.vector.tensor_mul(tmp, tmp2, tmp2)     # y^2
                nc.vector.tensor_mul(tmp, tmp, u)         # u*y^2
                nc.vector.tensor_scalar(
                    tmp, tmp, -0.5, 1.5, ALU.mult, ALU.add
                )                                         # 1.5 - u*y^2/2
                nc.vector.tensor_mul(
                    dst_bf if it == 1 else tmp2, tmp2, tmp
                )

        # ---- constants into SBUF ----
        cos_q = consts.tile([P, seq], BF16)
        sin_q = consts.tile([P, seq], BF16)
        nc.scalar.dma_start(cos_q[:], cosq[:])
        nc.scalar.dma_start(sin_q[:], sinq[:])
        if shared_cossin:
            cos_k, sin_k = cos_q, sin_q
        else:
            cos_k = consts.tile([P, seq], BF16, name="cos_k")
            sin_k = consts.tile([P, seq], BF16, name="sin_k")
            nc.sync.dma_start(cos_k[:], cosk[:])
            nc.sync.dma_start(sin_k[:], sink[:])
        w1_sb = consts.tile([P, 2, 512], xdt, name="w1")
        w2_sb = consts.tile([P, 2, 512], vgdt, name="w2")
        wo_sb = [consts.tile([P, N_EMBD], BF16, name=f"wo_{e}") for e in range(2)]
        nc.scalar.dma_start(w1_sb[:], w1t[:])
        nc.scalar.dma_start(w2_sb[:], w2t[:])
        for e in range(2):
            nc.scalar.dma_start(wo_sb[e][:], wot[e])
        mask_sb = consts.tile([P, 384], BF16)
        id_sb = consts.tile([P, P], BF16)
        nc.scalar.dma_start(mask_sb[:], maskm[:])
        nc.scalar.dma_start(id_sb[:], ident[:])
        onesP = consts.tile([P, 1], BF16)
        nc.scalar.dma_start(onesP[:], onesc[:, 0:1])
        ones256 = consts.tile([P, 256], BF16)
        nc.gpsimd.memset(ones256[:], 1.0)
        ones11 = consts.tile([1, 1], BF16)
        nc.gpsimd.memset(ones11[:], 1.0)
        expbias = consts.tile([P, 1], F32)
        nc.gpsimd.memset(expbias[:], -EXP_BIAS)

        def proj_mms(ps_out, x0_sb, w_sb, wsl, xsl):
            """Project: ps_out[P, width] += W[:, :, wsl].T @ x0[:, :, xsl],
            contracting the 256 emb dims (2 halves on dim1)."""
            if cfg.proj_fp8:
                nc.tensor.matmul(
                    ps_out, w_sb[:, :, wsl], x0_sb[:, :, xsl],
                    start=True, stop=True, perf_mode=DR,
                )
            else:
                for e in range(2):
                    nc.tensor.matmul(
                        ps_out, w_sb[:, e, wsl], x0_sb[:, e, xsl],
                        start=(e == 0), stop=(e == 1),
                    )

        def p1_units(b):
            """Units for P1(b): x0 load, per-(chunk,pair) proj+rope+sumsq,
            norm factors, scatter/broadcast + q/k scaling."""
            st = {"b": b}
            chunks = _chunks(seq, 512)

            def u_load():
                st["x0"] = x0p.tile([P, 2, seq], xdt, tag="x0", name="x0")
                nc.gpsimd.dma_start(st["x0"][:], x0t[b])
                if cfg.vg_fp8 and not cfg.proj_fp8:
                    st["x0f"] = x0p.tile(
                        [P, 2, seq], FP8, tag="x0f", name="x0f"
                    )
                    nc.gpsimd.dma_start(st["x0f"][:], x0f[b])
                else:
                    st["x0f"] = st["x0"]
                st["q"] = qkp.tile([P, 2, seq], BF16, tag="qk", name="qt")
                st["k"] = qkp.tile([P, 2, seq], BF16, tag="qk", name="kt")
                st["ss"] = smallp.tile(
                    [P, 4, nt], F32, tag="ssb", name="ssb", bufs=2
                )

            def u_pair(coff, cw, pair):
                t0 = coff // P
                sl = slice(coff, coff + cw)
                x0_sb = st["x0"]
                y, cc, ss = (
                    (st["q"], cos_q, sin_q),
                    (st["k"], cos_k, sin_k),
                )[pair]
                ps_c = psp.tile([P, 2, cw], F32, tag="c", name="ps_c",
                                bufs=1)
                for half in range(2):
                    s = 2 * pair + half
                    proj_mms(
                        ps_c[:, half, :], x0_sb, w1_sb,
                        slice(s * P, (s + 1) * P), sl,
                    )
                # evacuate raw projection to bf16 SBUF (Pool can't read
                # PSUM; alternate ACT/DVE to balance)
                raw = rawp.tile([P, 2, cw], BF16, tag="raw", name="raw")
                if (t0 + pair) % 2 == 0:
                    nc.scalar.copy(raw[:], ps_c[:])
                else:
                    nc.vector.tensor_copy(raw[:], ps_c[:])
                # squares on DVE bf16 (rotation preserves sumsq; use raw)
                sq = sqp.tile([P, 2, cw], BF16, tag="sq", name="sq")
                nc.vector.tensor_mul(sq[:], raw[:], raw[:])
                # per-position sumsq columns via N=1 ones-matmuls
                cwp = cw // P
                ps_sc = psp.tile([P, 2, cwp], F32, tag="c", name="ps_sc",
                                 bufs=1)
                for half in range(2):
                    for ti in range(cwp):
                        nc.tensor.matmul(
                            ps_sc[:, half, ti : ti + 1],
                            sq[:, half, ti * P : (ti + 1) * P],
                            onesP[:],
                            start=True,
                            stop=True,
                        )
                nc.vector.tensor_copy(
                    st["ss"][:, 2 * pair : 2 * pair + 2, t0 : t0 + cwp],
                    ps_sc[:],
                )
                # rope on DVE bf16 2x (raw -> roped halves of y)
                t1 = sqp.tile([P, cw], BF16, tag="ropet", name="t1")
                t2 = sqp.tile([P, cw], BF16, tag="ropet", name="t2")
                nc.vector.tensor_mul(t1[:], raw[:, 0, :], cc[:, sl])
                nc.vector.tensor_mul(t2[:], raw[:, 1, :], ss[:, sl])
                nc.vector.tensor_add(y[:, 0, sl], t1[:], t2[:])
                nc.vector.tensor_mul(t1[:], raw[:, 0, :], ss[:, sl])
                nc.vector.tensor_mul(t2[:], raw[:, 1, :], cc[:, sl])
                nc.vector.tensor_sub(y[:, 1, sl], t2[:], t1[:])

            def u_norm():
                # r = rsqrt(ss/(hd c^2) + eps/c^2) -> bf16, then scatter rows
                ss_sb = st["ss"]
                for row, s0, c2 in ((0, 0, cq2), (1, 2, ck2)):
                    u_t = smallp.tile([P, nt], F32, tag="ut", name="ut")
                    nw_t = smallp.tile([P, nt], F32, tag="nwt", name="nwt")
                    nw2 = smallp.tile([P, nt], F32, tag="nw2", name="nw2")
                    r_bf = smallp.tile([P, nt], BF16, tag="rbf", name="rbf")
                    nc.vector.tensor_add(
                        u_t[:], ss_sb[:, s0, :], ss_sb[:, s0 + 1, :]
                    )
                    nc.vector.tensor_scalar(
                        u_t[:], u_t[:], 1.0 / (HEAD_DIM * c2), eps / c2,
                        ALU.mult, ALU.add,
                    )
                    rsqrt_dve(r_bf[:], u_t[:], nw_t[:], nw2[:])
                    scat = bass.AP(
                        tensor=rq_dram.tensor,
                        offset=rq_dram[b, row].offset,
                        ap=[[1, P], [P, nt]],
                    )
                    nc.gpsimd.dma_start(scat, r_bf[:])

            def u_scale(coff, cw):
                # broadcast-load the factor rows and scale roped q and k
                sl = slice(coff, coff + cw)
                for row, z in ((0, st["q"]), (1, st["k"])):
                    r_b = rqp.tile([P, 512], BF16, tag="rqb", name="rqb")
                    bcast = bass.AP(
                        tensor=rq_dram.tensor,
                        offset=rq_dram[b, row].offset + coff,
                        ap=[[0, P], [1, cw]],
                    )
                    nc.scalar.dma_start(r_b[:, :cw], bcast)
                    nc.vector.tensor_mul(z[:, 0, sl], z[:, 0, sl], r_b[:, :cw])
                    nc.vector.tensor_mul(z[:, 1, sl], z[:, 1, sl], r_b[:, :cw])

            units = [u_load]
            for coff, cw in chunks:
                for pair in range(2):
                    units.append(
                        lambda coff=coff, cw=cw, pair=pair: u_pair(
                            coff, cw, pair
                        )
                    )
            units.append(u_norm)
            for coff, cw in _chunks(seq, 512):
                units.append(lambda coff=coff, cw=cw: u_scale(coff, cw))
            return st, units

        def p2_units(st):
            """Units for P2(b): per-tile v,g projection; v into PAIR tiles."""
            npair = (nt + 1) // 2
            st["v2"] = [None] * npair
            st["g"] = [None] * nt
            vg8 = cfg.proj_fp8 or cfg.vg_fp8
            # v evac scale: undo W_SCALE, apply V_SCALE (wot folds 1/V_SCALE)
            vs = (V_SCALE / W_SCALE) if vg8 else V_SCALE
            gs = (0.5 / W_SCALE) if vg8 else 0.5

            def u_tile(t):
                x0_sb = st["x0f"]
                tsl = slice(t * P, (t + 1) * P)
                ps_vg = psp.tile([P, 512], F32, tag="vg", name="ps_vg")
                if vg8:
                    nc.tensor.matmul(
                        ps_vg[:], x0_sb[:, :, tsl], w2_sb[:],
                        start=True, stop=True, perf_mode=DR,
                    )
                else:
                    for e in range(2):
                        nc.tensor.matmul(
                            ps_vg[:], x0_sb[:, e, tsl], w2_sb[:, e, :],
                            start=(e == 0), stop=(e == 1),
                        )
                if t % 2 == 0:
                    st["v2"][t // 2] = vp.tile(
                        [P, 2, 258], pvdt, tag="v", name="vt"
                    )
                vt = st["v2"][t // 2]
                half = t % 2
                nc.scalar.activation(
                    vt[:, half, 0:256], ps_vg[:, 0:256], AF.Copy, scale=vs
                )
                nc.gpsimd.memset(vt[:, half, 256:258], 1.0)
                gt = gp.tile([P, 256], BF16, tag="g", name="gt")
                # sigmoid(g) = 0.5*(1+tanh(g/2)); the 0.5 is folded into wot
                nc.scalar.activation(
                    gt[:], ps_vg[:, 256:512], AF.Tanh, scale=gs
                )
                nc.gpsimd.tensor_add(gt[:], gt[:], ones256[:])
                st["g"][t] = gt

            return [lambda t=t: u_tile(t) for t in range(nt)]

        def attn_units(st):
            """Units for attention(b): per-chunk kt-group S/exp/PV units and
            per-q-tile epilogue units."""
            b = st["b"]
            q_sb, k_sb = st["q"], st["k"]
            v2_sb, g_sb = st["v2"], st["g"]
            qchunks = [(t0c, min(2, nt - t0c)) for t0c in range(0, nt, 2)]
            qchunks.reverse()
            units = []
            for qc0, njt in qchunks:
                narrow = njt == 1 and qc0 == nt - 1 and t_real == qc0 * P
                qn = 1 if narrow else P
                qw = njt * qn
                qoff = qc0 * P
                cst = {}

                def u_group(grp, first, cst=cst, qc0=qc0, njt=njt, qw=qw,
                            qn=qn, qoff=qoff):
                    if first:
                        cst["ps_y"] = [
                            psp.tile([qn, 258], F32, tag="y", name="ps_y")
                            for _ in range(njt)
                        ]
                    ps_y = cst["ps_y"]
                    ng = len(grp)
                    ps_s = psp.tile([P, ng, qw], F32, tag="s", name="ps_s")
                    for i, kt in enumerate(grp):
                        diag = kt >= qc0
                        for e in range(2):
                            nc.tensor.matmul(
                                ps_s[:, i, :],
                                k_sb[:, e, kt * P : (kt + 1) * P],
                                q_sb[:, e, qoff : qoff + qw],
                                start=(e == 0),
                                stop=(e == 1) and not diag,
                            )
                        if diag:
                            # pre-exp causal mask: add NEG above diagonal
                            # (master slice puts tri at q-block d = kt-qc0)
                            d = kt - qc0
                            m0 = (1 - d) * P
                            nc.tensor.matmul(
                                ps_s[:, i, :],
                                id_sb[:],
                                mask_sb[:, m0 : m0 + qw],
                                start=False,
                                stop=True,
                            )
                    pt = ptp.tile([P, ng, qw], pvdt, tag="pt", name="pt")
                    nc.scalar.activation(pt[:], ps_s[:], AF.Exp,
                                         bias=expbias[:])
                    # PV
                    if cfg.pv_fp8:
                        pidx = grp[0] // 2
                        for j in range(njt):
                            jq = qc0 + j
                            if grp[0] > jq:
                                continue  # pair fully above diagonal
                            stop = jq // 2 == pidx
                            if ng == 2:
                                nc.tensor.matmul(
                                    ps_y[j][:],
                                    pt[:, :, j * P : (j + 1) * P],
                                    v2_sb[pidx][:],
                                    start=(pidx == 0),
                                    stop=stop,
                                    perf_mode=DR,
                                )
                            else:
                                nc.tensor.matmul(
                                    ps_y[j][:],
                                    pt[:, 0, j * P : (j + 1) * P],
                                    v2_sb[pidx][:, 0, :],
                                    start=(pidx == 0),
                                    stop=stop,
                                )
                    else:
                        for i, kt in enumerate(grp):
                            j0 = max(0, kt - qc0)
                            for j in range(j0, njt):
                                nc.tensor.matmul(
                                    ps_y[j][:],
                                    pt[:, i, j * qn : (j + 1) * qn],
                                    v2_sb[kt // 2][:, kt % 2, :],
                                    start=(kt == 0),
                                    stop=(kt == qc0 + j),
                                )

                def u_epi(j, cst=cst, qc0=qc0, qn=qn):
                    qt = qc0 + j
                    ps_y = cst["ps_y"]
                    rec = smallp.tile([qn, 1], F32, tag="rec", name="rec")
                    nc.vector.reciprocal_approx_fast(
                        rec[:], ps_y[j][:, 256:257]
                    )
                    yg = yp.tile([qn, 256], BF16, tag="yg", name="yg")
                    nc.vector.scalar_tensor_tensor(
                        yg[:], ps_y[j][:, 0:256], rec[:],
                        g_sb[qt][0:qn, :],
                        ALU.mult, ALU.mult,
                    )
                    ygt = ytp.tile([P, 2 * qn], BF16, tag="ygt", name="ygt")
                    if qn == P:
                        ps_t = psp.tile([P, 256], BF16, tag="y", name="ps_t")
                        for e in range(2):
                            nc.tensor.transpose(
                                ps_t[:, e * P : (e + 1) * P],
                                yg[:, e * P : (e + 1) * P],
                                id_sb[:],
                            )
                    else:
                        # 1-wide: transpose row via ones-rhs matmuls
                        # (plain matmul must write f32 PSUM)
                        ps_t = psp.tile([P, 2], F32, tag="y", name="ps_t")
                        for e in range(2):
                            nc.tensor.matmul(
                                ps_t[:, e : e + 1],
                                yg[0:1, e * P : (e + 1) * P],
                                ones11[:],
                            )
                    if qt % 2 == 0:
                        nc.scalar.copy(ygt[:], ps_t[:])
                    else:
                        nc.vector.tensor_copy(ygt[:], ps_t[:])
                    ps_o = psp.tile([qn, N_EMBD], F32, tag="y", name="ps_o")
                    for e in range(2):
                        nc.tensor.matmul(
                            ps_o[:],
                            ygt[:, e * qn : (e + 1) * qn],
                            wo_sb[e][:],
                            start=(e == 0),
                            stop=(e == 1),
                        )
                    o_sb = outp.tile([qn, N_EMBD], F32, tag="o", name="o_sb")
                    if qt % 2 == 0:
                        nc.vector.tensor_copy(o_sb[:], ps_o[:])
                    else:
                        nc.scalar.copy(o_sb[:], ps_o[:])
                    # DMA out, dropping the sink row (seq row 0)
                    r0 = qt * P - 1
                    p0 = 0
                    if qt == 0:
                        r0, p0 = 0, 1
                    rows = min(P - p0, t_real - r0)
                    if rows > 0:
                        nc.sync.dma_start(
                            out[b, r0 : r0 + rows, :], o_sb[p0 : p0 + rows, :]
                        )

                kts = list(range(qc0 + njt))
                for g0 in range(0, len(kts), 2):
                    units.append(
                        lambda grp=tuple(kts[g0 : g0 + 2]), first=(g0 == 0),
                        ug=u_group: ug(grp, first)
                    )
                for j in range(njt):
                    units.append(lambda j=j, ue=u_epi: ue(j))
            return units

        def interleave(*streams):
            tagged = []
            for si, s in enumerate(streams):
                n = len(s)
                for i, f in enumerate(s):
                    tagged.append(((i + 0.5) / n, si, i, f))
            tagged.sort(key=lambda t: (t[0], t[1], t[2]))
            for _, _, _, f in tagged:
                f()

        if cfg.depth2:
            prev, prev2 = None, None
            for rep in range(cfg.reps):
                for b in range(nb):
                    st, ua = p1_units(b)
                    ub = attn_units(prev2) if prev2 is not None else []
                    uc = p2_units(st)
                    interleave(ua, ub, uc)
                    prev2, prev = prev, st
            interleave(attn_units(prev2))
            interleave(attn_units(prev))
        else:
            prev = None
            for rep in range(cfg.reps):
                for b in range(nb):
                    st, ua = p1_units(b)
                    ub = attn_units(prev) if prev is not None else []
                    uc = p2_units(st)
                    interleave(ua, ub, uc)
                    prev = st
            interleave(attn_units(prev))

    nc.finalize()
    return nc


def _prep_core_inputs(cfg: Cfg, x, cos, sin, W_qkvg, W_sink, W_out, tao):
    """Host-side shard prep. Returns (shared dict, per-core list of dicts,
    cq2, shared_cossin, ck2)."""
    nb, seq, t_real = cfg.nb, cfg.seq, cfg.t_real
    x = np.asarray(x, np.float32)
    cos = np.asarray(cos, np.float32)
    sin = np.asarray(sin, np.float32)
    W_qkvg = np.asarray(W_qkvg, np.float32)
    W_sink = np.asarray(W_sink, np.float32)
    W_out = np.asarray(W_out, np.float32)
    tao = np.asarray(tao, np.float32)
    xdt = ml_dtypes.float8_e4m3 if cfg.proj_fp8 else ml_dtypes.bfloat16
    vg8 = cfg.proj_fp8 or cfg.vg_fp8
    vgdt = ml_dtypes.float8_e4m3 if vg8 else ml_dtypes.bfloat16
    wmul = W_SCALE if cfg.proj_fp8 else 1.0
    w2mul = W_SCALE if vg8 else 1.0

    cq = float(tao[0]) / np.sqrt(HEAD_DIM)
    ck = float(tao[1])
    sq_sign = 1.0 if cq >= 0 else -1.0
    sk_sign = 1.0 if ck >= 0 else -1.0
    cq2 = float(max(cq * cq, 1e-30))
    ck2 = float(max(ck * ck, 1e-30))
    shared_cossin = sq_sign == sk_sign

    # x0 padded: [nb, 128, 2, seq]
    x0t = np.zeros((nb, P, 2, seq), np.float32)
    for b in range(nb):
        x0 = np.concatenate([W_sink, x[b]], axis=0)  # [t_real+1, emb]
        buf = np.zeros((N_EMBD, seq), np.float32)
        buf[:, : x0.shape[0]] = x0.T
        x0t[b] = buf.reshape(2, P, seq).transpose(1, 0, 2)
    x0t = x0t.astype(xdt)

    ct = np.zeros((P, seq), np.float32)
    st = np.zeros((P, seq), np.float32)
    n_pos = min(cos.shape[0], seq)
    ct[:, :n_pos] = cos[:n_pos, 0, :].T
    st[:, :n_pos] = sin[:n_pos, 0, :].T

    # mask master: [NEG-full | tri-neg | zeros]
    trineg = np.where(
        np.tril(np.ones((P, P)), -1) > 0, np.float32(NEG), np.float32(0.0)
    )
    maskm = np.concatenate(
        [np.full((P, P), NEG, np.float32), trineg, np.zeros((P, P), np.float32)],
        axis=1,
    )

    shared = {
        "x0t": x0t,
        **(
            {"x0f": x0t.astype(ml_dtypes.float8_e4m3)}
            if (cfg.vg_fp8 and not cfg.proj_fp8)
            else {}
        ),
        "cosq": (sq_sign * ct).astype(ml_dtypes.bfloat16),
        "sinq": (sq_sign * st).astype(ml_dtypes.bfloat16),
        "maskm": maskm.astype(ml_dtypes.bfloat16),
        "onesc": np.ones((P, 2), ml_dtypes.bfloat16),
        "ident": np.eye(P).astype(ml_dtypes.bfloat16),
    }
    if not shared_cossin:
        shared["cosk"] = (sk_sign * ct).astype(ml_dtypes.bfloat16)
        shared["sink"] = (sk_sign * st).astype(ml_dtypes.bfloat16)

    per_core = []
    for h in range(N_CORES):
        r0 = 1024 * h
        w1 = wmul * W_qkvg[r0 : r0 + 512].T           # [256, 512] q|k
        w2 = w2mul * W_qkvg[r0 + 512 : r0 + 1024].T   # [256, 512] v|g
        # 0.5 from sigmoid; 1/V_SCALE undoes the stored-v scale
        wo = (0.5 / V_SCALE) * W_out[:, 256 * h : 256 * (h + 1)].T
        per_core.append(
            {
                "w1t": w1.reshape(2, P, 512).transpose(1, 0, 2).astype(xdt),
                "w2t": w2.reshape(2, P, 512).transpose(1, 0, 2).astype(vgdt),
                "wot": wo.reshape(2, P, N_EMBD).astype(ml_dtypes.bfloat16),
            }
        )
    return shared, per_core, cq2, shared_cossin, ck2


def kernel(x, cos, sin, W_qkvg, W_sink, W_out, tao, n_head):
    assert int(n_head) == N_HEAD
    cfg = FULL
    shared, per_core, cq2, shared_cossin, ck2 = _prep_core_inputs(
        cfg, x, cos, sin, W_qkvg, W_sink, W_out, tao
    )
    nc = build_program_cached(cfg, cq2, shared_cossin, ck2)

    in_maps = [dict(shared, **pc) for pc in per_core]
    from concourse.bass_utils import run_bass_kernel_spmd

    res = run_bass_kernel_spmd(nc, in_maps, core_ids=list(range(N_CORES)))
    total = np.zeros((cfg.nb, cfg.t_real, N_EMBD), np.float32)
    for r in res.results:
        total += r["out"]
    return total


_PROGRAM_CACHE = {}


def build_program_cached(cfg, cq2, shared_cossin, ck2):
    key = (cfg, round(cq2, 14), shared_cossin, round(ck2, 14))
    if key not in _PROGRAM_CACHE:
        _PROGRAM_CACHE[key] = build_program(cfg, cq2, shared_cossin, ck2)
    return _PROGRAM_CACHE[key]
